# revision 38
# baseline (speedup 1.0000x reference)
import sys

import numpy as np
import ml_dtypes

sys.path.insert(0, "/opt/trn_rl_repo")

import concourse.bass as bass  # noqa: E402
import concourse.bacc as bacc  # noqa: E402
import concourse.tile as tile  # noqa: E402
from concourse.tile import add_dep_helper  # noqa: E402
from concourse import mybir  # noqa: E402
from concourse.bass_utils import run_bass_kernel_spmd  # noqa: E402

# Problem dims (hardcoded per spec)
N, T, V, C_IN, C_OUT, K, KT = 256, 2048, 9, 16, 3, 5, 9
F_IN = V * C_IN    # 144
F_OUT = V * C_OUT  # 27
N_CORES = 8
NS = N // N_CORES  # 32 samples per core

WIN = 120          # conv output columns per window
NW = 18            # windows: 17*120 + 8 = 2048
WC = 128           # zT window length (WIN + 8 halo)
XCOLS = NS * WC + 4 * WC   # merged input tile: pose(32*128) | x2(4*128)
W17 = 16           # cols per sample in packed window 17 (12 data + 4 zero)
XC17 = NS * W17 + 4 * W17  # 576

F32 = mybir.dt.float32
F16 = mybir.dt.float16
F8 = mybir.dt.float8e3
E3 = ml_dtypes.float8_e3m4

_PROGRAM_CACHE = {}


def _build_program(warm=128, ztb=4, psg=4, psc=4, headq='sp', tailq=4, alt=False, late32=0, gate=0, xinb=NW-1, osbb=NW):
    nc = bacc.Bacc()

    # window-major merged input: xin[k, feat-row, :]:
    #   cols [0, 4096): pose feats 0..127, col 128*s + j, t = 120k - 4 + j
    #   cols [4096, 4608): feats 128..143 packed [16*(s%8)+c, 128*(s//8)+j]
    # zero-padded outside [0, T)
    xin = nc.declare_dram_parameter("xin", [NW - 1, 128, XCOLS], F8, isOutput=False)
    # window 17 packed: 16 cols/sample (t = 2036 + j, j<12 valid)
    xin17 = nc.declare_dram_parameter("xin17", [128, XC17], F8, isOutput=False)
    # packed consts: f16 = weff1(0:27) | w2big(27:243) | ball(243:1323)
    cf16 = nc.declare_dram_parameter("cf16", [128, 1323], F16, isOutput=False)
    # f16 = beff(0:432) | btcn(432:435)
    cf32 = nc.declare_dram_parameter("cf32", [128, 435], F16, isOutput=False)
    # raw dump: [window, time-in-window, 27*s + 3*w + o']; host unpacks
    out = nc.declare_dram_parameter("out", [NW, WIN, 27 * NS], F16, isOutput=True)

    with tile.TileContext(nc) as tc:
        with (
            tc.tile_pool(name="const", bufs=1) as cpool,
            tc.tile_pool(name="xin", bufs=xinb) as xp,
            tc.tile_pool(name="x17", bufs=1) as xp17,
            tc.tile_pool(name="zt", bufs=ztb) as ztp,
            tc.tile_pool(name="osb", bufs=osbb) as osp,
            tc.tile_pool(name="psG", bufs=psg, space=bass.MemorySpace.PSUM) as psG,
            tc.tile_pool(name="psC", bufs=psc, space=bass.MemorySpace.PSUM) as psC,
        ):
            cf16_sb = cpool.tile([128, 1323], F16, tag="cf16")
            cf32_sb = cpool.tile([128, 435], F16, tag="cf32")
            weff1_sb = cf16_sb[:, 0:27]
            w2big_sb = cf16_sb[:, 27:243]
            ball_sb = cf16_sb[:, 243:1323]
            beff_sb = cf32_sb[:, 0:432]
            btcn_sb = cf32_sb[:, 432:435]

            # PE warmup: keep the tensor engine busy (and the p-state ramp
            # running) while the first input/const DMAs are in flight
            wmt = cpool.tile([128, 64], F16, tag="wmt")
            nc.vector.memset(wmt[:], 0.0)
            psw = psG.tile([128, 432], F32, tag="g", name="warm")
            for i in range(warm):
                nc.tensor.matmul(psw[0:64, 0:64], wmt[:], wmt[:],
                                 start=True, stop=True)

            (nc.gpsimd if headq == 'pool' else nc.sync).dma_start(cf16_sb[:], cf16[:])
            x17t = xp17.tile([128, XC17], F8, tag="x17")
            nc.scalar.dma_start(x17t[:], xin17[:])
            if not late32:
                nc.scalar.dma_start(cf32_sb[:], cf32[:])
            xts, xdmas = [], []
            for k in range(NW - 1):
                xt = xp.tile([128, XCOLS], F8, tag="xin", name=f"x{k}")
                xdmas.append(nc.sync.dma_start(xt[:], xin[k]))
                xts.append(xt)
                if late32 and k == late32 - 1:
                    nc.scalar.dma_start(cf32_sb[:], cf32[:])

            for k in [NW - 1] + list(range(NW - 1)):
                last = k == NW - 1
                xt = x17t if last else xts[k]
                cw = W17 if last else WC   # cols per sample in this window
                x2base = NS * cw           # x2 region start
                nt = 12 if last else 128   # valid zT rows (time positions)

                # GCN: zT[t0 + p, 27*s + ch], t0 = 120k - 4
                zt = ztp.tile([128, 27 * NS], F16, tag="zt")
                if last:
                    nc.vector.memset(zt[:], 0.0)  # rows >= 12 stay 0 (t >= T)
                for h in range(2):  # 16 samples per psum bank
                    ps = psG.tile([128, 432], F32, tag="g")
                    for sl in range(16):
                        s = 16 * h + sl
                        nc.tensor.matmul(
                            ps[0:cw, 27 * sl:27 * sl + 27],
                            xt[:, cw * s:cw * s + cw], weff1_sb,
                            start=(sl == 0), stop=False,
                        )
                    for g2 in range(2):
                        g = 2 * h + g2
                        nc.tensor.matmul(
                            ps[0:cw, 216 * g2:216 * g2 + 216],
                            xt[:, x2base + cw * g:x2base + cw * g + cw], w2big_sb,
                            start=False, stop=(g2 == 1),
                        )
                    nc.vector.tensor_tensor(
                        zt[0:nt, 432 * h:432 * h + 432], ps[0:nt, :],
                        beff_sb[0:nt, :], mybir.AluOpType.add,
                    )
                if k == 0:
                    nc.gpsimd.memset(zt[0:4, :], 0.0)   # z[t<0] = 0

                # conv: out[120k + i, (w, o')] via banded-Toeplitz stationary
                ot = osp.tile([128, 27 * NS], F16, tag="osb")
                for op_ in range(3):
                    pc = psC.tile([128, 9 * NS], F32, tag="c")
                    for o in range(3):
                        q = 3 * o + op_
                        nc.tensor.matmul(
                            pc[0:WIN, :],
                            ball_sb[:, WIN * q:WIN * q + WIN],
                            zt[:, o:27 * NS:3],
                            start=(o == 0), stop=(o == 2),
                        )
                    nc.scalar.activation(
                        ot[0:WIN, op_:27 * NS:3], pc[0:WIN, :],
                        mybir.ActivationFunctionType.Lrelu,
                        bias=btcn_sb[0:WIN, op_:op_ + 1], alpha=0.01,
                    )
                nr = 8 if last else WIN
                if not last and k >= NW - 1 - tailq:
                    oeng = (nc.sync if (NW - 1 - k) % 2 == (1 if alt else 0)
                            else nc.gpsimd) if alt else nc.sync
                else:
                    oeng = nc.gpsimd
                oi = oeng.dma_start(out[k, 0:nr], ot[0:nr, :])
                if gate and not last:
                    add_dep_helper(oi.ins, xdmas[gate].ins,
                                   reason="defer outs behind input stream")

    nc.finalize()
    return nc


def _host_consts(A, W_gcn, b_gcn, W_tcn, b_tcn):
    A = np.asarray(A, np.float32)
    W_gcn = np.asarray(W_gcn, np.float32)
    b_gcn = np.asarray(b_gcn, np.float32)
    W_tcn = np.asarray(W_tcn, np.float32)
    b_tcn = np.asarray(b_tcn, np.float32)

    # W_eff[(v,c),(w,o)] = sum_k W_gcn[k,o,c] A[k,v,w]
    W_eff = np.einsum("koc,kvw->vcwo", W_gcn, A).reshape(F_IN, F_OUT)
    b_eff = np.einsum("ko,kw->wo", b_gcn, A.sum(axis=1)).reshape(F_OUT)

    cf16 = np.zeros((128, 1323), np.float16)
    cf16[:, 0:27] = W_eff[:128]
    for sm in range(8):
        cf16[16 * sm:16 * sm + 16, 27 + 27 * sm:27 + 27 * sm + 27] = W_eff[128:144]
    # conv taps: out[t,(w,o')] = sum_tau sum_o Ctaps[tau][o,o'] z[t+tau,(w,o)]
    Ctaps = {tau: W_tcn[:, :, 4 - tau, 0].T for tau in range(-4, 5)}
    ii = np.arange(WIN)
    for o in range(3):
        for op_ in range(3):
            q = 3 * o + op_
            for d in range(-4, 5):
                cf16[ii + d + 4, 243 + WIN * q + ii] = Ctaps[d][o, op_]

    cf32 = np.zeros((128, 435), np.float16)
    cf32[:, 0:432] = np.tile(b_eff, 16)[None, :]
    cf32[:, 432:435] = b_tcn[None, :]
    return cf16, cf32


def _host_windows(pose):
    # pose [N, T, 144] f32 -> per-core window-major fp8 arrays
    x8 = np.ascontiguousarray(pose.transpose(0, 2, 1)).astype(E3)  # [N, 144, T]
    Q = np.zeros((N, F_IN, 2304), E3)
    Q[:, :, 4:4 + T] = x8
    sN, sF, sT = Q.strides
    Wv = np.lib.stride_tricks.as_strided(
        Q, shape=(N, F_IN, NW, WC), strides=(sN, sF, 120 * sT, sT))
    xins, x17s = [], []
    for c in range(N_CORES):
        Wc = Wv[32 * c:32 * c + 32]                       # [32, 144, 18, 128]
        xin = np.zeros((NW - 1, 128, XCOLS), E3)
        xin[:, :, :NS * WC] = (
            Wc[:, :128, :NW - 1].transpose(2, 1, 0, 3).reshape(NW - 1, 128, NS * WC))
        w2 = Wc[:, 128:144, :NW - 1].reshape(4, 8, 16, NW - 1, WC)
        xin[:, :, NS * WC:] = (
            w2.transpose(3, 1, 2, 0, 4).reshape(NW - 1, 128, 4 * WC))
        # window 17 packed: 16 cols per sample, t = 2036 + j (j < 12 valid)
        x17 = np.zeros((128, XC17), E3)
        x17[:, :NS * W17] = (
            Wc[:, :128, NW - 1, :W17].transpose(1, 0, 2).reshape(128, NS * W17))
        w217 = Wc[:, 128:144, NW - 1, :W17].reshape(4, 8, 16, W17)
        x17[:, NS * W17:] = (
            w217.transpose(1, 2, 0, 3).reshape(128, 4 * W17))
        xins.append(xin)
        x17s.append(x17)
    return xins, x17s


def _run(inputs, **spmd_kwargs):
    pose = np.asarray(inputs["pose_feats"], np.float32)
    xins, x17s = _host_windows(pose)
    cf16, cf32 = _host_consts(
        inputs["A"], inputs["W_gcn"], inputs["b_gcn"], inputs["W_tcn"], inputs["b_tcn"]
    )

    if "prog" not in _PROGRAM_CACHE:
        _PROGRAM_CACHE["prog"] = _build_program()
    nc = _PROGRAM_CACHE["prog"]

    in_maps = []
    for i in range(N_CORES):
        in_maps.append({
            "xin": xins[i], "xin17": x17s[i], "cf16": cf16, "cf32": cf32,
        })
    res = run_bass_kernel_spmd(nc, in_maps, list(range(N_CORES)), **spmd_kwargs)
    outs = [res.results[i]["out"] for i in range(N_CORES)]
    full = np.stack(outs, axis=0)                 # [8, 18, 120, 864]
    full = full.reshape(N_CORES, NW, WIN, NS, F_OUT)
    full = full.transpose(0, 3, 1, 2, 4).reshape(N, NW * WIN, F_OUT)
    return full[:, :T].astype(np.float32), res


def kernel(**inputs) -> np.ndarray:
    out, _ = _run(inputs)
    return out
